# revision 48
# baseline (speedup 1.0000x reference)
"""Contrastive loss kernel for Trainium2, 8 NeuronCores (SPMD, raw Bass).

Math: loss*n = sum_{pos pairs}(1 - s) + sum_{neg pairs, s > 0.5} s over
s = x @ x.T with x [4096, 1024] L2-normalized and targets = arange(4096)//8
(classes are contiguous 8-row blocks, so the same-class mask is block-diagonal).

Distribution: sim is symmetric, so only the "upper triangle" of 256-row
chunk-pairs is computed: 16 chunks, core c owns row-chunks c and c+8 and the
chunk-pairs (c, c+d) for d=0..8 plus (c+8, c+8+d) for d=0..7 — every unordered
chunk-pair exactly once across the 8 cores, ~53% of the full matmul.  Each
core receives a ROTATED copy of x^T (rolled by 256*c embedding rows) so the
program is identical on every core; only the input data differs.

Scheduling:
  - chunks are DMA'd in order [4,0,5,1,6,2,7,3] on the sync HWDGE (sc4
    first: it holds the b-row lhsT columns and unlocks the first tile pair;
    sc0 second: it unlocks the a-diag group and, with sc4, the two final
    256-wide tiles, giving the late chunks slack against the feed).  The
    four diagonal tiles run first (t0-t3), the two 256-wide tiles last to
    shrink the tail.
  - the DMA feed delivers one chunk every ~1.3-2.3us while the PE consumes
    one every ~1.73us, so the stream start is deliberately delayed (warmup
    count) until the arrival frontier is ahead of the consumption schedule:
    a PE semaphore wait that blocks costs ~0.5-1.2us of wake-up latency, so
    waits should find their semaphore already satisfied.
  - HAM warmup matmuls (on a zeroed tile; results land in ps[7], later
    overwritten by tile 7's start=True) bridge the PE busy-streak into the
    real stream so the clock gate is at 8/8 throughout: any >=0.5us idle
    gap between warmups and stream can poison a free-running activity
    window and delay the 2.4GHz flip by up to ~3.4us of half-clock.

Elementwise split (sum over s > margin decomposed per tile):
  sum_{s>m} s = sum relu(s - m) + m * count(s > m)
  - plain tiles: the scalar engine's PSUM->SBUF copy *is* the relu (bias
    -0.5) with a fused per-partition accumulate (R); the DVE does one
    tensor_scalar is_gt count with accumulate (C).  The last two tiles'
    counts read raw s straight from PSUM (banks 0/1, never reused) so they
    run concurrently with ACT's relu in the tail.
  - diagonal tiles: scalar copies raw s; DVE computes g = s * (s > m) with
    accumulate (G), a combined correction C01 = sum over the diagonal half
    of g * (1 + same8), and the positive-pair strip sum A = sum s * (same8
    - eye).
The output DMA is split: acc cols 0:42 go out once 16 tiles are done on
both elementwise engines; the last two tiles' four accum cells (42:46)
follow in a tiny second DMA.  Host combines the per-core [128, 46]
partials:  loss = (28672 + sum(2*G + 2*R + C - C01 - A)) / 4096.
"""

import numpy as np
import ml_dtypes

import concourse.bass as bass
import concourse.mybir as mybir
from concourse.bass_utils import run_bass_kernel_spmd

N = 4096
D = 1024
NCORES = 8
CH = 256  # chunk = 256 embedding rows; 16 chunks
MARGIN = 0.5
KT = 8  # contraction tiles of 128
F32 = mybir.dt.float32
BF16 = mybir.dt.bfloat16
F8 = mybir.dt.float8e4  # e4m3
ALU = mybir.AluOpType
AFT = mybir.ActivationFunctionType

# rowpart -> (super-chunk holding its lhsT columns, column offset within it)
_ROWPARTS = {"a0": (0, 0), "a1": (0, 128), "b0": (4, 0), "b1": (4, 128)}

# (rowpart, rhs super-chunk, width, diag side or None) — diag groups at
# t0/t1 and t4/t5 interleaved with plain groups, 256-wide tiles last.
_TILES = [
    ("b0", 4, 512, "L"),
    ("b1", 4, 512, "R"),
    ("a0", 0, 512, "L"),
    ("a1", 0, 512, "R"),
    ("b0", 5, 512, None),
    ("b1", 5, 512, None),
    ("a0", 1, 512, None),
    ("a1", 1, 512, None),
    ("b0", 6, 512, None),
    ("b1", 6, 512, None),
    ("a0", 2, 512, None),
    ("a1", 2, 512, None),
    ("b0", 7, 512, None),
    ("b1", 7, 512, None),
    ("a0", 3, 512, None),
    ("a1", 3, 512, None),
    ("a0", 4, 256, None),  # block (a, a+8) — small tiles last to shrink tail
    ("a1", 4, 256, None),
]
NT = len(_TILES)  # 18
_DIAG_T = [0, 1, 2, 3]  # tile indices of the diagonal tiles, in order
NPS = 8  # psum ring slots (= all 8 banks)
NSB = 10  # s_sb ring slots (slack so the DVE diag burst never stalls ACT)
NWARM = 9  # HAM warmup matmuls.  Cold warmups pace the sequencer at ~427ns
# each, so it reaches the first-chunk wait at ~warmup_start + (NWARM-2)*427ns.
# Tuned so that point lands at the median first-chunk sem post: the wait then
# passes without blocking (a blocked PE wait has ~0.5-1.2us wake-up latency)
# while queued warmups keep the array busy — zero idle, so the HAM busy-streak
# that flips the clock gate to 8/8 is never poisoned.  An idle gap here costs
# up to a full free-running activity window (~3.4us) of half-clock.

# chunk DMA issue order on sync = exact consumption order.  sc0 is second:
# it unlocks both the a-diag group and (with sc4) the two final 256-wide
# tiles, so the late chunks gain ~2 tiles of slack against the feed.
_CHUNK_ORDER = [4, 0, 5, 1, 6, 2, 7, 3]
# tile index -> chunk first needed at that tile (waited at tile start)
_CHUNK_NEEDED_AT = {2: 0, 4: 5, 6: 1, 8: 6, 10: 2, 12: 7, 14: 3}


def _build_nc():
    nc = bass.Bass()
    # [super-chunk, partition, k, col] — each super-chunk slice is a fully
    # contiguous 0.5 MiB so HW-DGE descriptors are 8 KiB per partition.
    xTr = nc.declare_dram_parameter("xTr", [8, 128, KT, 512], F8, isOutput=False)
    # mask values are 0/1/2 — exact in fp8, halving the masks' HBM traffic
    masks = nc.declare_dram_parameter("masks", [128, 1024], F8, isOutput=False)
    out = nc.declare_dram_parameter("out", [128, 46], F32, isOutput=True)

    import contextlib

    with contextlib.ExitStack() as ctx:
        sc = [
            ctx.enter_context(nc.sbuf_tensor(f"sc{j}", [128, KT, 512], F8))
            for j in range(8)
        ]
        masks_sb = ctx.enter_context(nc.sbuf_tensor("masks_sb", [128, 1024], F8))
        s_sb = [
            ctx.enter_context(nc.sbuf_tensor(f"s{i}", [128, 512], BF16))
            for i in range(NSB)
        ]
        g_sb = [
            ctx.enter_context(nc.sbuf_tensor(f"g{i}", [128, 512], BF16))
            for i in range(2)
        ]
        g_plain = ctx.enter_context(nc.sbuf_tensor("g_plain", [128, 512], BF16))
        scrA = ctx.enter_context(nc.sbuf_tensor("scrA", [128, 256], BF16))
        scrB = ctx.enter_context(nc.sbuf_tensor("scrB", [128, 256], BF16))
        warm_sb = ctx.enter_context(nc.sbuf_tensor("warm_sb", [128, 512], BF16))
        bias_sb = ctx.enter_context(nc.sbuf_tensor("bias_sb", [128, 1], F32))
        # accumulator: 0:16 per-tile G (diag) / R (plain), 18:22 C01,
        # 22:26 A, 26:44 per-tile count C (plain tiles), 44:46 R of the
        # last two tiles (kept at the end for the split output DMA)
        acc = ctx.enter_context(nc.sbuf_tensor("acc", [128, 46], F32))

        ps = [
            ctx.enter_context(nc.psum_tensor(f"ps{i}", [128, 512], F32))
            for i in range(NPS)
        ]

        sem_sc = [ctx.enter_context(nc.semaphore(f"sem_sc{j}")) for j in range(8)]
        sem_mask = ctx.enter_context(nc.semaphore("sem_mask"))
        init_sem = ctx.enter_context(nc.semaphore("init_sem"))
        sem_out = ctx.enter_context(nc.semaphore("sem_out"))
        mm_sem = ctx.enter_context(nc.semaphore("mm_sem"))
        act_sem = ctx.enter_context(nc.semaphore("act_sem"))
        dve_sem = ctx.enter_context(nc.semaphore("dve_sem"))

        block = ctx.enter_context(nc.Block())

        @block.gpsimd
        def _(gpsimd):
            # zero the warmup tile first: matmuls on uninitialized SBUF can
            # contain NaN/Inf bit patterns that have crashed the exec unit
            gpsimd.memset(warm_sb[:], 0.0).then_inc(init_sem, 1)
            gpsimd.memset(bias_sb[:], -MARGIN).then_inc(init_sem, 1)
            # masks via SWDGE: slow but off the critical feed queues; issued
            # immediately — delaying it until after the first chunk lands
            # measures ~3.8us WORSE (the DVE's diagonal burst blocks on the
            # masks and the backlog cascades through the s_sb ring into ACT
            # and the PE's psum-ring waits)
            gpsimd.dma_start(masks_sb[:], masks[:]).then_inc(sem_mask, 16)

        @block.sync
        def _(sync):
            # chunks in exact consumption order so the PE never outruns the
            # feed (splitting chunks into smaller DMAs measurably slows the
            # overall feed — per-DMA overhead delays every later chunk)
            for j in _CHUNK_ORDER:
                sync.dma_start(sc[j][:], xTr[j]).then_inc(sem_sc[j], 16)
            # split output: the bulk of acc is stable once 16 tiles are done
            # on both elementwise engines; only the last two tiles' four
            # accum cells (C16,C17 @42:44 and R16,R17 @44:46) arrive later
            sync.wait_ge(dve_sem, NT - 2)
            sync.wait_ge(act_sem, NT - 2)
            sync.dma_start(out[:, 0:42], acc[:, 0:42]).then_inc(sem_out, 16)
            sync.wait_ge(dve_sem, NT)
            sync.wait_ge(act_sem, NT)
            # no completion wait: the framework's end-of-program drain on the
            # sync engine covers the in-flight output DMAs
            sync.dma_start(out[:, 42:46], acc[:, 42:46]).then_inc(sem_out, 16)

        @block.tensor
        def _(tensor):
            # HAM warmup (results discarded; ps[7] is overwritten by tile
            # 7's start=True): bridges the PE busy-streak into the real
            # stream so the clock gate is already at 8/8.
            tensor.wait_ge(init_sem, 1)
            for _ in range(NWARM):
                tensor.matmul(
                    ps[NPS - 1][:, 0:512],
                    warm_sb[:, 0:128],
                    warm_sb[:],
                    start=True,
                    stop=True,
                )
            tensor.wait_ge(sem_sc[4], 16)
            for t, (rp, j, w, _side) in enumerate(_TILES):
                lsc, moff = _ROWPARTS[rp]
                if t in _CHUNK_NEEDED_AT:
                    tensor.wait_ge(sem_sc[_CHUNK_NEEDED_AT[t]], 16)
                if t >= NPS:
                    tensor.wait_ge(act_sem, t - (NPS - 1))
                pst = ps[t % NPS]
                mm = None
                for kp in range(KT // 2):
                    mm = tensor.matmul(
                        pst[:, 0:w],
                        sc[lsc][:, 2 * kp : 2 * kp + 2, moff : moff + 128],
                        sc[j][:, 2 * kp : 2 * kp + 2, 0:w],
                        start=(kp == 0),
                        stop=(kp == KT // 2 - 1),
                        perf_mode=mybir.MatmulPerfMode.DoubleRow,
                    )
                mm.then_inc(mm_sem, 1)

        @block.scalar
        def _(scalar):
            scalar.wait_ge(init_sem, 2)
            for t, (_rp, _j, w, side) in enumerate(_TILES):
                scalar.wait_ge(mm_sem, t + 1)
                if t >= NSB:
                    scalar.wait_ge(dve_sem, t - NSB + 1)
                if side is None:
                    # the PSUM->SBUF copy *is* the relu: r = relu(s - m),
                    # accumulating R = sum relu(s - m) on the fly.  The last
                    # two tiles' R lands in cols 44/45 so the bulk of acc is
                    # stable early for the split output DMA.
                    rcol = 44 + (t - (NT - 2)) if t >= NT - 2 else t
                    scalar.activation(
                        s_sb[t % NSB][:, 0:w],
                        ps[t % NPS][:, 0:w],
                        AFT.Relu,
                        bias=bias_sb[:, 0:1],
                        accum_out=acc[:, rcol : rcol + 1],
                    ).then_inc(act_sem, 1)
                else:
                    # diag tiles need raw s for the masked corrections
                    scalar.copy(
                        s_sb[t % NSB][:, 0:w], ps[t % NPS][:, 0:w]
                    ).then_inc(act_sem, 1)

        @block.vector
        def _(vector):
            d_idx = 0
            mask_waited = False
            for t, (_rp, _j, w, side) in enumerate(_TILES):
                if t >= NT - 2:
                    # PSUM-direct count: gate on the matmuls, not the copy
                    # (banks 6,7,0,1 of tiles 14-17 are never reused)
                    vector.wait_ge(mm_sem, t + 1)
                else:
                    vector.wait_ge(act_sem, t + 1)
                s_t = s_sb[t % NSB]
                if side is None:
                    if t >= NT - 2:
                        # last two tiles: count straight from PSUM (banks
                        # 0/1, never reused) so it runs concurrently with
                        # ACT's relu — shortest possible tail
                        vector.tensor_scalar(
                            out=g_plain[:, 0:w],
                            in0=ps[t % NPS][:, 0:w],
                            scalar1=MARGIN,
                            scalar2=1.0,
                            op0=ALU.is_gt,
                            op1=ALU.mult,
                            accum_out=acc[:, 26 + t : 27 + t],
                        ).then_inc(dve_sem, 1)
                    else:
                        # count C = sum [s > m] == sum [relu(s - m) > 0]
                        vector.tensor_scalar(
                            out=g_plain[:, 0:w],
                            in0=s_t[:, 0:w],
                            scalar1=0.0,
                            scalar2=1.0,
                            op0=ALU.is_gt,
                            op1=ALU.mult,
                            accum_out=acc[:, 26 + t : 27 + t],
                        ).then_inc(dve_sem, 1)
                else:
                    g_t = g_sb[d_idx % 2]
                    # full-width G including the same-class strip; the C01
                    # correction below removes the strip's contribution
                    vector.scalar_tensor_tensor(
                        out=g_t[:, 0:512],
                        in0=s_t[:, 0:512],
                        scalar=MARGIN,
                        in1=s_t[:, 0:512],
                        op0=ALU.is_gt,
                        op1=ALU.mult,
                        accum_out=acc[:, t : t + 1],
                    )
                    vector.drain()  # next op reads g_t written just above
                    if not mask_waited:
                        # first masked op: masks only gate from here on, so
                        # the unmasked g-op above can start before they land
                        vector.wait_ge(sem_mask, 16)
                        mask_waited = True
                    aoff = 0 if side == "L" else 256
                    soff = 0 if side == "L" else 128
                    moff2 = 512 if side == "L" else 896
                    # C01 = sum over diag half of g * (1 + same8):
                    # host subtracts it once, which turns 2*G_full into
                    # 2*(off-diag half) + 1*(diag half without same-class)
                    vector.scalar_tensor_tensor(
                        out=scrA[:],
                        in0=g_t[:, 0:256],
                        scalar=1.0,
                        in1=masks_sb[:, aoff : aoff + 256],
                        op0=ALU.mult,
                        op1=ALU.mult,
                        accum_out=acc[:, 18 + d_idx : 19 + d_idx],
                    )
                    # positive-pair sum over the 128-wide same-class strip
                    vector.scalar_tensor_tensor(
                        out=scrB[:, 0:128],
                        in0=s_t[:, soff : soff + 128],
                        scalar=1.0,
                        in1=masks_sb[:, moff2 : moff2 + 128],
                        op0=ALU.mult,
                        op1=ALU.mult,
                        accum_out=acc[:, 22 + d_idx : 23 + d_idx],
                    ).then_inc(dve_sem, 1)
                    d_idx += 1

    return nc


_NC_CACHE = None


def _get_nc():
    global _NC_CACHE
    if _NC_CACHE is None:
        _NC_CACHE = _build_nc()
    return _NC_CACHE


def _host_masks():
    m8 = (np.arange(128)[:, None] // 8 == np.arange(128)[None, :] // 8).astype(
        np.float32
    )
    ma = m8 - np.eye(128, dtype=np.float32)
    masks = np.zeros((128, 1024), np.float32)
    masks[:, 0:128] = 1.0 + m8  # C01-L  (cols 128:256 stay 1)
    masks[:, 128:256] = 1.0
    masks[:, 256:384] = 1.0  # C01-R
    masks[:, 384:512] = 1.0 + m8
    masks[:, 512:640] = ma  # maskA left strip
    masks[:, 640:768] = 0.0
    masks[:, 768:896] = 0.0
    masks[:, 896:1024] = ma  # maskA right strip
    return masks.astype(ml_dtypes.float8_e4m3)


def kernel(inputs: np.ndarray, targets: np.ndarray) -> np.ndarray:
    x = np.asarray(inputs, dtype=np.float32)
    assert x.shape == (N, D)
    # [128, 8, 4096] fp8 e4m3: xTr[p, k, n] = x[n, k*128 + p]
    xTr = np.ascontiguousarray(x.T.reshape(KT, 128, N).transpose(1, 0, 2)).astype(
        ml_dtypes.float8_e4m3
    )
    masks = _host_masks()
    in_maps = []
    for c in range(NCORES):
        xc = np.roll(xTr, -CH * c, axis=2)
        # [j, p, k, c] with each super-chunk j contiguous
        xc = np.ascontiguousarray(
            xc.reshape(128, KT, 8, 512).transpose(2, 0, 1, 3)
        )
        in_maps.append({"xTr": xc, "masks": masks})

    nc = _get_nc()
    res = run_bass_kernel_spmd(nc, in_maps, core_ids=list(range(NCORES)))

    plain_t = [t for t in range(NT) if t not in _DIAG_T]
    total = 0.0
    for c in range(NCORES):
        o = np.asarray(res.results[c]["out"], dtype=np.float64)
        # 2*(G for diag, R for plain); last two tiles' R live in cols 44:46
        total += 2.0 * (o[:, 0 : NT - 2].sum() + o[:, 44:46].sum())
        total += MARGIN * 2.0 * sum(o[:, 26 + t].sum() for t in plain_t)  # counts
        total -= o[:, 18:22].sum()  # C01
        total -= o[:, 22:26].sum()  # A
    # positive-pair count: 4 regions/core * 128 rows * 7 partners * 8 cores
    loss = (28672.0 + total) / float(N)
    return np.float32(loss)


# revision 54
# speedup vs baseline: 1.0169x; 1.0169x over previous
"""Contrastive loss kernel for Trainium2, 8 NeuronCores (SPMD, raw Bass).

Math: loss*n = sum_{pos pairs}(1 - s) + sum_{neg pairs, s > 0.5} s over
s = x @ x.T with x [4096, 1024] L2-normalized and targets = arange(4096)//8
(classes are contiguous 8-row blocks, so the same-class mask is block-diagonal).

Distribution: sim is symmetric, so only the "upper triangle" of 256-row
chunk-pairs is computed: 16 chunks, core c owns row-chunks c and c+8 and the
chunk-pairs (c, c+d) for d=0..8 plus (c+8, c+8+d) for d=0..7 — every unordered
chunk-pair exactly once across the 8 cores, ~53% of the full matmul.  Each
core receives a ROTATED copy of x^T (rolled by 256*c embedding rows) so the
program is identical on every core; only the input data differs.

Scheduling:
  - chunks are DMA'd in order [4,0,5,1,6,2,7,3] on the sync HWDGE (sc4
    first: it holds the b-row lhsT columns and unlocks the first tile pair;
    sc0 second: it unlocks the a-diag group and, with sc4, the two final
    256-wide tiles, giving the late chunks slack against the feed).  The
    four diagonal tiles run first (t0-t3), the two 256-wide tiles last to
    shrink the tail.
  - the DMA feed delivers one chunk every ~1.3-2.3us while the PE consumes
    one every ~1.73us, so the stream start is deliberately delayed (warmup
    count) until the arrival frontier is ahead of the consumption schedule:
    a PE semaphore wait that blocks costs ~0.5-1.2us of wake-up latency, so
    waits should find their semaphore already satisfied.
  - HAM warmup matmuls (on a zeroed tile; results land in ps[7], later
    overwritten by tile 7's start=True) bridge the PE busy-streak into the
    real stream so the clock gate is at 8/8 throughout: any >=0.5us idle
    gap between warmups and stream can poison a free-running activity
    window and delay the 2.4GHz flip by up to ~3.4us of half-clock.

Elementwise split (sum over s > margin decomposed per tile):
  sum_{s>m} s = sum relu(s - m) + m * count(s > m)
  - plain tiles: the scalar engine's PSUM->SBUF copy *is* the relu (bias
    -0.5) with a fused per-partition accumulate (R); the DVE does one
    tensor_scalar is_gt count with accumulate (C).  The last two tiles'
    counts read raw s straight from PSUM (banks 0/1, never reused) so they
    run concurrently with ACT's relu in the tail.
  - diagonal tiles: scalar copies raw s; DVE computes g = s * (s > m) with
    accumulate (G), a combined correction C01 = sum over the diagonal half
    of g * (1 + same8), and the positive-pair strip sum A = sum s * (same8
    - eye).
The output DMA is split: acc cols 0:42 go out once 16 tiles are done on
both elementwise engines; the last two tiles' four accum cells (42:46)
follow in a tiny second DMA.  Host combines the per-core [128, 46]
partials:  loss = (28672 + sum(2*G + 2*R + C - C01 - A)) / 4096.
"""

import numpy as np
import ml_dtypes

import concourse.bass as bass
import concourse.mybir as mybir
from concourse.bass_utils import run_bass_kernel_spmd

N = 4096
D = 1024
NCORES = 8
CH = 256  # chunk = 256 embedding rows; 16 chunks
MARGIN = 0.5
KT = 8  # contraction tiles of 128
F32 = mybir.dt.float32
BF16 = mybir.dt.bfloat16
F8 = mybir.dt.float8e4  # e4m3
ALU = mybir.AluOpType
AFT = mybir.ActivationFunctionType

# rowpart -> (super-chunk holding its lhsT columns, column offset within it)
_ROWPARTS = {"a0": (0, 0), "a1": (0, 128), "b0": (4, 0), "b1": (4, 128)}

# (rowpart, rhs super-chunk, width, diag side or None) — diag groups at
# t0/t1 and t4/t5 interleaved with plain groups, 256-wide tiles last.
_TILES = [
    ("b0", 4, 512, "L"),
    ("b1", 4, 512, "R"),
    ("a0", 0, 512, "L"),
    ("a1", 0, 512, "R"),
    ("b0", 5, 512, None),
    ("b1", 5, 512, None),
    ("a0", 1, 512, None),
    ("a1", 1, 512, None),
    ("b0", 6, 512, None),
    ("b1", 6, 512, None),
    ("a0", 2, 512, None),
    ("a1", 2, 512, None),
    ("b0", 7, 512, None),
    ("b1", 7, 512, None),
    ("a0", 3, 512, None),
    ("a1", 3, 512, None),
    ("a0", 4, 256, None),  # block (a, a+8) — small tiles last to shrink tail
    ("a1", 4, 256, None),
]
NT = len(_TILES)  # 18
_DIAG_T = [0, 1, 2, 3]  # tile indices of the diagonal tiles, in order
NPS = 8  # psum ring slots (= all 8 banks)
NSB = 10  # s_sb ring slots (slack so the DVE diag burst never stalls ACT)
NWARM = 9  # HAM warmup matmuls.  Cold warmups pace the sequencer at ~427ns
# each, so it reaches the first-chunk wait at ~warmup_start + (NWARM-2)*427ns.
# Tuned so that point lands at the median first-chunk sem post: the wait then
# passes without blocking (a blocked PE wait has ~0.5-1.2us wake-up latency)
# while queued warmups keep the array busy — zero idle, so the HAM busy-streak
# that flips the clock gate to 8/8 is never poisoned.  An idle gap here costs
# up to a full free-running activity window (~3.4us) of half-clock.

# chunk DMA issue order on sync = exact consumption order.  sc0 is second:
# it unlocks both the a-diag group and (with sc4) the two final 256-wide
# tiles, so the late chunks gain ~2 tiles of slack against the feed.
_CHUNK_ORDER = [4, 0, 5, 1, 6, 2, 7, 3]
# tile index -> chunk first needed at that tile (waited at tile start)
_CHUNK_NEEDED_AT = {2: 0, 4: 5, 6: 1, 8: 6, 10: 2, 12: 7, 14: 3}


def _build_nc():
    nc = bass.Bass()
    # [super-chunk, partition, k, col] — each super-chunk slice is a fully
    # contiguous 0.5 MiB so HW-DGE descriptors are 8 KiB per partition.
    xTr = nc.declare_dram_parameter("xTr", [8, 128, KT, 512], F8, isOutput=False)
    # mask values are 0/1/2 — exact in fp8, halving the masks' HBM traffic
    masks = nc.declare_dram_parameter("masks", [128, 1024], F8, isOutput=False)
    out = nc.declare_dram_parameter("out", [128, 46], F32, isOutput=True)

    import contextlib

    with contextlib.ExitStack() as ctx:
        sc = [
            ctx.enter_context(nc.sbuf_tensor(f"sc{j}", [128, KT, 512], F8))
            for j in range(8)
        ]
        masks_sb = ctx.enter_context(nc.sbuf_tensor("masks_sb", [128, 1024], F8))
        s_sb = [
            ctx.enter_context(nc.sbuf_tensor(f"s{i}", [128, 512], BF16))
            for i in range(NSB)
        ]
        g_sb = [
            ctx.enter_context(nc.sbuf_tensor(f"g{i}", [128, 512], BF16))
            for i in range(2)
        ]
        g_plain = ctx.enter_context(nc.sbuf_tensor("g_plain", [128, 512], BF16))
        scrA = ctx.enter_context(nc.sbuf_tensor("scrA", [128, 256], BF16))
        scrB = ctx.enter_context(nc.sbuf_tensor("scrB", [128, 256], BF16))
        warm_sb = ctx.enter_context(nc.sbuf_tensor("warm_sb", [128, 512], BF16))
        bias_sb = ctx.enter_context(nc.sbuf_tensor("bias_sb", [128, 1], F32))
        # accumulator: 0:16 per-tile G (diag) / R (plain), 18:22 C01,
        # 22:26 A, 26:44 per-tile count C (plain tiles), 44:46 R of the
        # last two tiles (kept at the end for the split output DMA)
        acc = ctx.enter_context(nc.sbuf_tensor("acc", [128, 46], F32))

        ps = [
            ctx.enter_context(nc.psum_tensor(f"ps{i}", [128, 512], F32))
            for i in range(NPS)
        ]

        sem_sc = [ctx.enter_context(nc.semaphore(f"sem_sc{j}")) for j in range(8)]
        sem_mask = ctx.enter_context(nc.semaphore("sem_mask"))
        init_sem = ctx.enter_context(nc.semaphore("init_sem"))
        sem_out = ctx.enter_context(nc.semaphore("sem_out"))
        mm_sem = ctx.enter_context(nc.semaphore("mm_sem"))
        act_sem = ctx.enter_context(nc.semaphore("act_sem"))
        dve_sem = ctx.enter_context(nc.semaphore("dve_sem"))

        block = ctx.enter_context(nc.Block())

        @block.gpsimd
        def _(gpsimd):
            # zero the warmup tile first: matmuls on uninitialized SBUF can
            # contain NaN/Inf bit patterns that have crashed the exec unit
            gpsimd.memset(warm_sb[:], 0.0).then_inc(init_sem, 1)
            gpsimd.memset(bias_sb[:], -MARGIN).then_inc(init_sem, 1)
            # masks via SWDGE: slow but off the critical feed queues; issued
            # immediately — delaying it until after the first chunk lands
            # measures ~3.8us WORSE (the DVE's diagonal burst blocks on the
            # masks and the backlog cascades through the s_sb ring into ACT
            # and the PE's psum-ring waits)
            gpsimd.dma_start(masks_sb[:], masks[:]).then_inc(sem_mask, 16)

        @block.sync
        def _(sync):
            # chunks in exact consumption order so the PE never outruns the
            # feed (splitting chunks into smaller DMAs measurably slows the
            # overall feed — per-DMA overhead delays every later chunk)
            for j in _CHUNK_ORDER:
                sync.dma_start(sc[j][:], xTr[j]).then_inc(sem_sc[j], 16)
            # split output: the bulk of acc is stable once 16 tiles are done
            # on both elementwise engines; only the last two tiles' four
            # accum cells (C16,C17 @42:44 and R16,R17 @44:46) arrive later
            sync.wait_ge(dve_sem, NT - 2)
            sync.wait_ge(act_sem, NT - 2)
            sync.dma_start(out[:, 0:42], acc[:, 0:42]).then_inc(sem_out, 16)
            sync.wait_ge(dve_sem, NT)
            sync.wait_ge(act_sem, NT)
            # no completion wait: the framework's end-of-program drain on the
            # sync engine covers the in-flight output DMAs
            sync.dma_start(out[:, 42:46], acc[:, 42:46]).then_inc(sem_out, 16)

        @block.tensor
        def _(tensor):
            # HAM warmup (results discarded; ps[7] is overwritten by tile
            # 7's start=True): bridges the PE busy-streak into the real
            # stream so the clock gate is already at 8/8.
            tensor.wait_ge(init_sem, 1)
            for _ in range(NWARM):
                tensor.matmul(
                    ps[NPS - 1][:, 0:512],
                    warm_sb[:, 0:128],
                    warm_sb[:],
                    start=True,
                    stop=True,
                )
            tensor.wait_ge(sem_sc[4], 16)
            for t, (rp, j, w, _side) in enumerate(_TILES):
                lsc, moff = _ROWPARTS[rp]
                if t in _CHUNK_NEEDED_AT:
                    tensor.wait_ge(sem_sc[_CHUNK_NEEDED_AT[t]], 16)
                if t >= NPS:
                    tensor.wait_ge(act_sem, t - (NPS - 1))
                pst = ps[t % NPS]
                mm = None
                for kp in range(KT // 2):
                    mm = tensor.matmul(
                        pst[:, 0:w],
                        sc[lsc][:, 2 * kp : 2 * kp + 2, moff : moff + 128],
                        sc[j][:, 2 * kp : 2 * kp + 2, 0:w],
                        start=(kp == 0),
                        stop=(kp == KT // 2 - 1),
                        perf_mode=mybir.MatmulPerfMode.DoubleRow,
                    )
                mm.then_inc(mm_sem, 1)

        @block.scalar
        def _(scalar):
            scalar.wait_ge(init_sem, 2)
            for t, (_rp, _j, w, side) in enumerate(_TILES):
                scalar.wait_ge(mm_sem, t + 1)
                if t >= NSB:
                    scalar.wait_ge(dve_sem, t - NSB + 1)
                if side is None:
                    # the PSUM->SBUF copy *is* the relu: r = relu(s - m),
                    # accumulating R = sum relu(s - m) on the fly.  The last
                    # two tiles' R lands in cols 44/45 so the bulk of acc is
                    # stable early for the split output DMA.
                    rcol = 44 + (t - (NT - 2)) if t >= NT - 2 else t
                    scalar.activation(
                        s_sb[t % NSB][:, 0:w],
                        ps[t % NPS][:, 0:w],
                        AFT.Relu,
                        bias=bias_sb[:, 0:1],
                        accum_out=acc[:, rcol : rcol + 1],
                    ).then_inc(act_sem, 1)
                else:
                    # diag tiles need raw s for the masked corrections
                    scalar.copy(
                        s_sb[t % NSB][:, 0:w], ps[t % NPS][:, 0:w]
                    ).then_inc(act_sem, 1)

        @block.vector
        def _(vector):
            d_idx = 0
            mask_waited = False
            # NOTE: processing the PSUM-direct t16/t17 counts ahead of
            # t14/t15 (to post dve_sem=NT earlier) crashes the exec unit —
            # same hazard class as PSUM-direct reads on banks 6/7.  Keep
            # strict tile order.
            for t, (_rp, _j, w, side) in enumerate(_TILES):
                if t >= NT - 2:
                    # PSUM-direct count: gate on the matmuls, not the copy
                    # (banks 6,7,0,1 of tiles 14-17 are never reused)
                    vector.wait_ge(mm_sem, t + 1)
                else:
                    vector.wait_ge(act_sem, t + 1)
                s_t = s_sb[t % NSB]
                if side is None:
                    ccol = 26 + t
                    if t >= NT - 2:
                        # last two tiles: count straight from PSUM (banks
                        # 0/1, never reused) so it runs concurrently with
                        # ACT's relu — shortest possible tail
                        vector.tensor_scalar(
                            out=g_plain[:, 0:w],
                            in0=ps[t % NPS][:, 0:w],
                            scalar1=MARGIN,
                            scalar2=1.0,
                            op0=ALU.is_gt,
                            op1=ALU.mult,
                            accum_out=acc[:, ccol : ccol + 1],
                        ).then_inc(dve_sem, 1)
                    else:
                        # count C = sum [s > m] == sum [relu(s - m) > 0]
                        vector.tensor_scalar(
                            out=g_plain[:, 0:w],
                            in0=s_t[:, 0:w],
                            scalar1=0.0,
                            scalar2=1.0,
                            op0=ALU.is_gt,
                            op1=ALU.mult,
                            accum_out=acc[:, ccol : ccol + 1],
                        ).then_inc(dve_sem, 1)
                else:
                    g_t = g_sb[d_idx % 2]
                    # full-width G including the same-class strip; the C01
                    # correction below removes the strip's contribution
                    vector.scalar_tensor_tensor(
                        out=g_t[:, 0:512],
                        in0=s_t[:, 0:512],
                        scalar=MARGIN,
                        in1=s_t[:, 0:512],
                        op0=ALU.is_gt,
                        op1=ALU.mult,
                        accum_out=acc[:, t : t + 1],
                    )
                    vector.drain()  # next op reads g_t written just above
                    if not mask_waited:
                        # first masked op: masks only gate from here on, so
                        # the unmasked g-op above can start before they land
                        vector.wait_ge(sem_mask, 16)
                        mask_waited = True
                    aoff = 0 if side == "L" else 256
                    soff = 0 if side == "L" else 128
                    moff2 = 512 if side == "L" else 896
                    # C01 = sum over diag half of g * (1 + same8):
                    # host subtracts it once, which turns 2*G_full into
                    # 2*(off-diag half) + 1*(diag half without same-class)
                    vector.scalar_tensor_tensor(
                        out=scrA[:],
                        in0=g_t[:, 0:256],
                        scalar=1.0,
                        in1=masks_sb[:, aoff : aoff + 256],
                        op0=ALU.mult,
                        op1=ALU.mult,
                        accum_out=acc[:, 18 + d_idx : 19 + d_idx],
                    )
                    # positive-pair sum over the 128-wide same-class strip
                    vector.scalar_tensor_tensor(
                        out=scrB[:, 0:128],
                        in0=s_t[:, soff : soff + 128],
                        scalar=1.0,
                        in1=masks_sb[:, moff2 : moff2 + 128],
                        op0=ALU.mult,
                        op1=ALU.mult,
                        accum_out=acc[:, 22 + d_idx : 23 + d_idx],
                    ).then_inc(dve_sem, 1)
                    d_idx += 1

    return nc


_NC_CACHE = None


def _get_nc():
    global _NC_CACHE
    if _NC_CACHE is None:
        _NC_CACHE = _build_nc()
    return _NC_CACHE


def _host_masks():
    m8 = (np.arange(128)[:, None] // 8 == np.arange(128)[None, :] // 8).astype(
        np.float32
    )
    ma = m8 - np.eye(128, dtype=np.float32)
    masks = np.zeros((128, 1024), np.float32)
    masks[:, 0:128] = 1.0 + m8  # C01-L  (cols 128:256 stay 1)
    masks[:, 128:256] = 1.0
    masks[:, 256:384] = 1.0  # C01-R
    masks[:, 384:512] = 1.0 + m8
    masks[:, 512:640] = ma  # maskA left strip
    masks[:, 640:768] = 0.0
    masks[:, 768:896] = 0.0
    masks[:, 896:1024] = ma  # maskA right strip
    return masks.astype(ml_dtypes.float8_e4m3)


def kernel(inputs: np.ndarray, targets: np.ndarray) -> np.ndarray:
    x = np.asarray(inputs, dtype=np.float32)
    assert x.shape == (N, D)
    # [128, 8, 4096] fp8 e4m3: xTr[p, k, n] = x[n, k*128 + p]
    xTr = np.ascontiguousarray(x.T.reshape(KT, 128, N).transpose(1, 0, 2)).astype(
        ml_dtypes.float8_e4m3
    )
    masks = _host_masks()
    in_maps = []
    for c in range(NCORES):
        xc = np.roll(xTr, -CH * c, axis=2)
        # [j, p, k, c] with each super-chunk j contiguous
        xc = np.ascontiguousarray(
            xc.reshape(128, KT, 8, 512).transpose(2, 0, 1, 3)
        )
        in_maps.append({"xTr": xc, "masks": masks})

    nc = _get_nc()
    res = run_bass_kernel_spmd(nc, in_maps, core_ids=list(range(NCORES)))

    plain_t = [t for t in range(NT) if t not in _DIAG_T]
    total = 0.0
    for c in range(NCORES):
        o = np.asarray(res.results[c]["out"], dtype=np.float64)
        # 2*(G for diag, R for plain); last two tiles' R live in cols 44:46
        total += 2.0 * (o[:, 0 : NT - 2].sum() + o[:, 44:46].sum())
        total += MARGIN * 2.0 * sum(o[:, 26 + t].sum() for t in plain_t)  # counts
        total -= o[:, 18:22].sum()  # C01
        total -= o[:, 22:26].sum()  # A
    # positive-pair count: 4 regions/core * 128 rows * 7 partners * 8 cores
    loss = (28672.0 + total) / float(N)
    return np.float32(loss)
